# revision 14
# baseline (speedup 1.0000x reference)
"""Trainium2 Bass kernel for nn_Net_67954972557347 (dense_mlp).

Network: a1 = lrelu(a@Wa+ba) [B,68]; b1 = lrelu(b@Wb+bb) [B,68];
c = [a1|b1|meta] [B,140]; then 10 lrelu'd dense layers
(140->34->34->20->20->20->20->20->5->2->1), lrelu slope 0.01.

Strategy: pure data parallel over 8 cores (32768 rows each), activations
feature-major ([feat, batch]); batch streams 512 columns per pipeline
step through the PE (fp32r datapath).

6 matmuls / 2 PSUM tiles / 2 drain groups per step:
  psT (3 banks, 1536 cols):
    cols 0:512    bankE [c0; c2; c4; c6; c8; ones] <- MM3(T1h) + MM4(B1)
                                                      + MM5(TOh)
    cols 512:1024 bankO [c1; c3; c5; c7; y; ones]  <- MM6(TEh)
    cols 1024:1536 bank1 [a1; meta; ones]          <- MM1(t1)
  ps2 (1 bank): bank2 [b1; ones]                   <- MM2(t2)
t1 = [a.T; ilrelu(meta); ones], t2 = [b.T; ones] are the DMA streams.
The even/odd chain banks advance all ten tail layers in two
block-diagonal matmuls. ALL biases are folded in-PSUM via ones-row
passthrough columns, so drains are pure leaky-relu: psT drains in ONE
1536-col ACT Prelu into the TEO tile (whose three 512-col halves are
the next step's matmul rhs windows); ps2 drains on DVE (copy +
max(0.01x, x); PSUM cannot be a dual stt operand).

EVERY matmul uses K=128 (full-partition rhs window, zero weight rows
beyond the real contraction) and M>=69: the PE array reconfigures its
tile geometry whenever round-up(K)/round-up(M) changes between
consecutive matmuls, which locks the clock at the mid p-state (0.83
ns/col instead of 0.42 — measured 427 vs 229 ns per 512-col matmul).
Uniform 128x128 tiles keep it at full speed for free (cost scales with
the moving dim only). SBUF operand buffers are fixed, self-managed
rings, fully memset once so the padded partition rows multiply as 0.0
(never NaN garbage).

Latency hiding: matmuls read tiles drained TWO steps ago (age-2) and
input DMA is prefetched two steps ahead, so the PE's in-order queue
never waits on same-step drains. The t2 stream is padded to an even
partition count: odd-partition DMAs land on a single DMA queue instead
of spreading across all 16. Pipeline depth 2 steps/layer * 10 stages
= 20 steps.
"""

import os
import sys

import numpy as np

for _p in ("/opt/trn_rl_repo", "/root/.axon_site/_ro/trn_rl_repo"):
    if os.path.isdir(_p) and _p not in sys.path:
        sys.path.append(_p)

import concourse.bass as bass
import concourse.mybir as mybir
import concourse.tile as tile
from concourse import bacc
from concourse.bass_utils import run_bass_kernel_spmd
from bass_rust import add_dep_helper

F32 = mybir.dt.float32
F32R = mybir.dt.float32r
ALU = mybir.AluOpType
PRELU = mybir.ActivationFunctionType.Prelu

B_FULL = 262144
N_CORES = 8
B_CORE = B_FULL // N_CORES          # 32768
N = 512                              # columns per chunk (fp32 PSUM bank)
PIPE = 20                            # 10 stages x 2-step latency
AGE = 2                              # drain-to-consume latency in steps
ALPHA = 0.01                         # leaky-relu slope

# partition row counts
K1 = 50          # t1: a(45) + ilrelu(meta)(4) + ones(1)
K2 = 104         # t2: b(102) + ones(1) + zero pad (even row
                 # count: odd-partition DMAs pin to one queue)
MT = 128         # matmul M / drain partitions: padded to the full 128 so
                 # every drain also rewrites the pad rows with lrelu(0)=0 —
                 # no startup memset needed for the activation buffers
M2 = MT

# weight tile column spans (every matmul M=128)
CM1, CM2, CM3, CM4, CM5, CM6 = 0, 128, 256, 384, 512, 640
WT_COLS = 768

NB_IN = 6        # t1/t2 buffer ring depth
NB_ACT = 4       # teo/b1 buffer ring depth


def _ilrelu(x):
    """Inverse of leaky-relu (slope 0.01)."""
    return np.where(x > 0, x, x * (1.0 / ALPHA)).astype(np.float32)


def _pack_weights(Wa, ba, Wb, bb, Ws, Bs):
    """Build the [128, WT_COLS] packed weight tile (biases via ones rows)."""
    W0, W1, W2, W3, W4, W5, W6, W7, W8, W9 = Ws
    B0, B1, B2, B3, B4, B5, B6, B7, B8, B9 = Bs
    wt = np.zeros((128, WT_COLS), np.float32)
    # MM1: rhs t1 -> bank1 [a1(0:68); meta(68:72); ones(72)]
    c = CM1
    wt[0:45, c:c + 68] = Wa
    wt[45:49, c + 68:c + 72] = np.eye(4, dtype=np.float32)
    wt[49, c:c + 68] = ba
    wt[49, c + 72] = 1.0
    # MM2: rhs t2 -> bank2 [b1(0:68); ones(68)]
    c = CM2
    wt[0:102, c:c + 68] = Wb
    wt[102, c:c + 68] = bb
    wt[102, c + 68] = 1.0
    # MM3: rhs T1h -> bankE c0 part (cols 0:34) + ones (col 96)
    c = CM3
    wt[0:68, c:c + 34] = W0[0:68]
    wt[68:72, c:c + 34] = W0[136:140]
    wt[72, c:c + 34] = B0
    wt[72, c + 96] = 1.0
    # MM4: rhs B1 -> bankE c0 part (cols 0:34)
    c = CM4
    wt[0:68, c:c + 34] = W0[68:136]
    # MM5: rhs TOh = [c1;c3;c5;c7;y;ones] -> bankE evens
    c = CM5
    wt[0:34, c + 34:c + 54] = W2    # c1 -> c2
    wt[34:54, c + 54:c + 74] = W4   # c3 -> c4
    wt[54:74, c + 74:c + 94] = W6   # c5 -> c6
    wt[74:79, c + 94:c + 96] = W8   # c7 -> c8
    wt[80, c + 34:c + 54] = B2
    wt[80, c + 54:c + 74] = B4
    wt[80, c + 74:c + 94] = B6
    wt[80, c + 94:c + 96] = B8
    # MM6: rhs TEh = [c0;c2;c4;c6;c8;ones] -> bankO odds
    c = CM6
    wt[0:34, c:c + 34] = W1         # c0 -> c1
    wt[34:54, c + 34:c + 54] = W3   # c2 -> c3
    wt[54:74, c + 54:c + 74] = W5   # c4 -> c5
    wt[74:94, c + 74:c + 79] = W7   # c6 -> c7
    wt[94:96, c + 79:c + 80] = W9   # c8 -> y
    wt[96, c:c + 34] = B1
    wt[96, c + 34:c + 54] = B3
    wt[96, c + 54:c + 74] = B5
    wt[96, c + 74:c + 79] = B7
    wt[96, c + 79] = B9[0]
    wt[96, c + 80] = 1.0
    return wt


def _pack_core_inputs(a, b, meta, n_chunks):
    """Pack one core's shard into the t1/t2 DMA streams."""
    bc = n_chunks * N
    t1 = np.empty((K1, bc), np.float32)
    t1[0:45] = a[:bc].T
    t1[45:49] = _ilrelu(meta[:bc].T)
    t1[49] = 1.0
    t2 = np.zeros((K2, bc), np.float32)
    t2[0:102] = b[:bc].T
    t2[102] = 1.0
    return t1, t2


def build_bass(n_chunks):
    """Build + compile the per-core Bass program (same on all 8 cores)."""
    nc = bacc.Bacc(None, target_bir_lowering=False, debug=False)
    n_steps = n_chunks + PIPE

    t1_d = nc.dram_tensor("t1", [K1, n_chunks * N], F32, kind="ExternalInput")
    t2_d = nc.dram_tensor("t2", [K2, n_chunks * N], F32, kind="ExternalInput")
    wt_d = nc.dram_tensor("wt", [128, WT_COLS], F32, kind="ExternalInput")
    y_d = nc.dram_tensor("y", [1, n_chunks * N], F32, kind="ExternalOutput")

    with tile.TileContext(nc) as tc:
        with (
            tc.tile_pool(name="const", bufs=1) as constp,
            tc.tile_pool(name="psT", bufs=2, space=bass.MemorySpace.PSUM) as psTp,
            tc.tile_pool(name="ps2", bufs=2, space=bass.MemorySpace.PSUM) as ps2p,
        ):
            wt = constp.tile([128, WT_COLS], F32R, tag="wt")
            nc.sync.dma_start(wt[:], wt_d[:].bitcast(F32R))

            # fixed operand buffers, fully zeroed once: writers only touch
            # the live partition rows, so rows above stay 0.0 forever and
            # the K=128 rhs windows multiply clean zeros
            t1b = [constp.tile([128, N], F32R, tag=f"t1b{i}",
                                name=f"t1b{i}") for i in range(NB_IN)]
            t2b = [constp.tile([128, N], F32R, tag=f"t2b{i}",
                                name=f"t2b{i}") for i in range(NB_IN)]
            teob = [constp.tile([128, 3 * N], F32R, tag=f"teob{i}",
                                 name=f"teob{i}") for i in range(NB_ACT)]
            b1b = [constp.tile([128, N], F32R, tag=f"b1b{i}",
                                name=f"b1b{i}") for i in range(NB_ACT)]
            # the [0:128] drains rewrite every teob/b1b row each step, so
            # only the two generations read before any drain (ages -2/-1 =
            # ring slots 2 and 3) need zeroing; t1b/t2b need their padded
            # partition rows zeroed once. Spread the memsets over three
            # engines to shorten startup.
            init_tiles = t1b + t2b + [teob[2], teob[3], b1b[2], b1b[3]]
            for i, tl in enumerate(init_tiles):
                eng = nc.gpsimd if i % 2 == 0 else nc.vector
                eng.memset(tl[:].bitcast(F32), 0.0)

            def w(c0, m):
                return wt[0:128, c0:c0 + m]

            def chain(*insts):
                for i in range(1, len(insts)):
                    add_dep_helper(insts[i].ins, insts[i - 1].ins,
                                   sync=False, reason="psum acc order")

            def dma_in(c):
                # t1 on the SP DGE, t2 on the Pool DGE: two dma_starts per
                # step saturate a single sequencer's descriptor generation
                # (~800ns each) and stall the PE's input waits
                if c < n_chunks:
                    nc.sync.dma_start(
                        t1b[c % NB_IN][0:K1],
                        t1_d[:, c * N:(c + 1) * N].bitcast(F32R))
                    nc.gpsimd.dma_start(
                        t2b[c % NB_IN][0:K2],
                        t2_d[:, c * N:(c + 1) * N].bitcast(F32R))

            for c in range(AGE + 2):
                dma_in(c)

            for t in range(n_steps):
                dma_in(t + AGE + 2)
                mm = nc.tensor.matmul

                teo = teob[(t - AGE) % NB_ACT]
                b1 = b1b[(t - AGE) % NB_ACT]
                # tail steps (t >= n_chunks) skip stage 1 entirely: the
                # chain then consumes STALE bank1/bank2 drains (finite, and
                # their ones rows still carry the bias passthrough); those
                # chunks are never output
                real = t < n_chunks

                psT = psTp.tile([128, 3 * N], F32, tag="psT", name=f"psT_{t}")
                ps2 = ps2p.tile([128, N], F32, tag="ps2", name=f"ps2_{t}")

                # ---- chain banks first: their deps are 2 steps old ----
                i1 = mm(psT[0:MT, 0:N], w(CM3, MT), teo[0:128, 2 * N:3 * N],
                        start=True, stop=False, tile_position=(0, 0))
                i2 = mm(psT[0:MT, 0:N], w(CM4, MT), b1[0:128],
                        start=False, stop=False, tile_position=(0, 0))
                i3 = mm(psT[0:MT, 0:N], w(CM5, MT), teo[0:128, N:2 * N],
                        start=False, stop=True, tile_position=(0, 0))

                chain(i1, i2, i3)

                mm(psT[0:MT, N:2 * N], w(CM6, MT), teo[0:128, 0:N],
                   start=True, stop=True, tile_position=(0, 0))

                # ---- stage 1 ----
                if real:
                    mm(psT[0:MT, 2 * N:3 * N], w(CM1, MT),
                       t1b[t % NB_IN][0:128],
                       start=True, stop=True, tile_position=(0, 0))
                    mm(ps2[0:M2], w(CM2, M2), t2b[t % NB_IN][0:128],
                       start=True, stop=True, tile_position=(0, 0))

                # ---- drains (pure lrelu; biases already in PSUM) ----
                teo_t = teob[t % NB_ACT]
                nc.scalar.activation(teo_t[0:MT, 0:3 * N], psT[0:MT],
                                     PRELU, alpha=ALPHA)
                if real:
                    b1_t = b1b[t % NB_ACT]
                    nc.vector.tensor_copy(b1_t[0:M2], ps2[0:M2])
                    nc.vector.scalar_tensor_tensor(
                        b1_t[0:M2], b1_t[0:M2], ALPHA, b1_t[0:M2],
                        ALU.mult, ALU.max)

                # ---- y out (row 79 of the odd half) ----
                if t >= PIPE:
                    c = t - PIPE
                    nc.gpsimd.dma_start(
                        y_d[:, c * N:(c + 1) * N].bitcast(F32R),
                        teo_t[79:80, N:2 * N])

    nc.compile()
    return nc


_NC_CACHE = {}


def _get_nc(n_chunks):
    if n_chunks not in _NC_CACHE:
        _NC_CACHE[n_chunks] = build_bass(n_chunks)
    return _NC_CACHE[n_chunks]


def run_cores(inputs, n_chunks, cores, trace=False, trace_kwargs=None):
    """Pack inputs, run the SPMD kernel on the given cores, return
    (per-core y arrays, BassKernelResults)."""
    a = np.asarray(inputs["a"], np.float32)
    b = np.asarray(inputs["b"], np.float32)
    meta = np.asarray(inputs["meta"], np.float32)
    Ws = [np.asarray(inputs[f"W{i}"], np.float32) for i in range(10)]
    Bs = [np.asarray(inputs[f"B{i}"], np.float32) for i in range(10)]
    wt = _pack_weights(np.asarray(inputs["Wa"], np.float32),
                       np.asarray(inputs["ba"], np.float32),
                       np.asarray(inputs["Wb"], np.float32),
                       np.asarray(inputs["bb"], np.float32), Ws, Bs)
    in_maps = []
    for r in cores:
        sl = slice(r * B_CORE, r * B_CORE + n_chunks * N)
        t1, t2 = _pack_core_inputs(a[sl], b[sl], meta[sl], n_chunks)
        in_maps.append({"t1": t1, "t2": t2, "wt": wt})
    nc = _get_nc(n_chunks)
    kw = dict(trace=trace)
    if trace_kwargs:
        kw.update(trace_kwargs)
    res = run_bass_kernel_spmd(nc, in_maps, list(range(len(cores))), **kw)
    return [res.results[i]["y"] for i in range(len(cores))], res


def kernel(**inputs):
    n_chunks = B_CORE // N
    ys, _ = run_cores(inputs, n_chunks, list(range(N_CORES)))
    out = np.empty((B_FULL, 1), np.float32)
    for r in range(N_CORES):
        out[r * B_CORE:(r + 1) * B_CORE, 0] = ys[r][0]
    return out
